# revision 15
# baseline (speedup 1.0000x reference)
"""VQ codebook nearest-neighbor encode kernel for Trainium2 (8 NeuronCores).

Pipeline (matches the reference nn.Module):
  conv3x3(SAME) -> positionwise linear -> argmin_k |x - e_k|^2.

Algebraic folds (host-side, fp64):
  * The positionwise linear commutes with the conv:
      L @ conv(W, x) == conv(L @ W, x)
    so the linear layer is fused into the conv weights and disappears.
  * argmin_k |x-e_k|^2 = argmax_k (x.e_k - 0.5|e_k|^2); the conv/linear
    biases shift every x by one constant vector c, so they fold into the
    per-codeword table eb2'_k = 0.5|e_k|^2 - c.e_k. No on-chip bias adds.

On-chip compute:
  * Conv as 9 shifted matmuls accumulating in PSUM (H padded to 10 in SBUF,
    4D strided moving APs, per-element has_written semantics with the full
    (0,0) shift first). cin-chunk-outer loop over all 8 PSUM banks so
    matmuls start as soon as the first latent chunk lands.
  * Scores: per 512-codeword chunk, first a K=2 matmul injects -eb2'
    (ones x [eb2_hi; eb2_lo]), then 6 product matmuls accumulate.
  * All matmuls are fp16 hi/lo 3-pass products (a.b ~= ah.bh + ah.bl + al.bh,
    fp32 PSUM accumulation): ~2^-24 relative error at 1 cycle/row (vs 4 for
    native fp32), so the argmin matches the fp32 reference exactly.
  * ScalarE persists score chunks PSUM->SBUF; VectorE finds the argmax with
    max8 + max_index (first-occurrence ties, like jnp.argmin).

Sharding: data-parallel over batch. Each core processes 32 images (2048
positions) end-to-end with a full copy of the weights and codebook. No
collectives; the host concatenates the 8 index shards.
"""

import sys

sys.path.insert(0, "/opt/trn_rl_repo")

import numpy as np

import concourse.bass as bass  # noqa: F401  (registers engines)
import concourse.tile as tile
from concourse import bacc, mybir
from concourse.bass_utils import run_bass_kernel_spmd

F16 = mybir.dt.float16
F32 = mybir.dt.float32
U32 = mybir.dt.uint32
I32 = mybir.dt.int32

B, CIN, COUT, H, W, K = 256, 512, 256, 8, 8, 8192
NCORES = 8
BL = B // NCORES          # images per core (32)
NPOS = BL * H * W         # positions per core (2048)
NPC = NPOS // 128         # pos-chunks per core (16)
NKC = K // 512            # codebook chunks (16)

_CACHE = {}


def build_kernel(n_iters: int = 1):
    nc = bacc.Bacc("TRN2", target_bir_lowering=False, debug=False,
                   num_devices=NCORES)
    aps = {}
    for nm, shape, dt in [
        ("lat_h", [BL, CIN, H, W], F16), ("lat_l", [BL, CIN, H, W], F16),
        ("wt_h", [3, 3, CIN, COUT], F16), ("wt_l", [3, 3, CIN, COUT], F16),
        ("emb_h", [COUT, K], F16), ("emb_l", [COUT, K], F16),
        ("eb2hl", [2, K], F16),
    ]:
        aps[nm] = nc.dram_tensor(nm, shape, dt, kind="ExternalInput").ap()
    idx_ap = nc.dram_tensor("idx", [NPOS], I32, kind="ExternalOutput").ap()

    with tile.TileContext(nc) as tc:
        for _ in range(n_iters):
            _emit_body(nc, tc, aps, idx_ap)
    nc.compile()
    return nc


def _emit_body(nc, tc, aps, idx_ap):
    from contextlib import ExitStack
    ctx = ExitStack()
    with ctx:
        glob = ctx.enter_context(tc.tile_pool(name="glob", bufs=1))
        psp = ctx.enter_context(tc.tile_pool(name="ps", bufs=8, space="PSUM"))

        # ---- whole-kernel-lifetime tiles ----
        emb = {}
        for p in ("h", "l"):
            for d in range(2):
                emb[p, d] = glob.tile([128, K], F16, name=f"emb_{p}{d}",
                                      tag=f"emb_{p}{d}")
        eb2t = glob.tile([2, K], F16, name="eb2t")
        ones2 = glob.tile([2, 128], F16, name="ones2")
        nc.vector.memset(ones2[:], 1.0)

        flat = {}   # (p, dout_chunk) -> [128, NPOS] fp16 fused conv+linear out
        for p in ("h", "l"):
            for d in range(2):
                flat[p, d] = glob.tile([128, NPOS], F16, name=f"flat_{p}{d}",
                                       tag=f"flat_{p}{d}")
        idx_all = glob.tile([128, NPC], U32, name="idx_all")

        # ---- phase 1: fused conv (9 shifted matmuls, H padded to 10) ----
        shifts = [(0, 0)] + [(dy, dx) for dy in (-1, 0, 1) for dx in (-1, 0, 1)
                             if (dy, dx) != (0, 0)]
        prods = [("h", "h"), ("h", "l"), ("l", "h")]  # (weight, latent)
        with tc.tile_pool(name="p1", bufs=1) as p1:
            latp, wts = {}, {}
            lat_src = {p: aps[f"lat_{p}"].rearrange("b c h w -> c b h w")
                       for p in ("h", "l")}
            wt_src = {p: aps[f"wt_{p}"].rearrange("ky kx ci co -> ci (ky kx) co")
                      for p in ("h", "l")}
            for c in range(4):          # per-cin-chunk loads, in loop order
                for p in ("h", "l"):
                    t = p1.tile([128, BL, 10, 8], F16, name=f"latp_{p}{c}",
                                tag=f"latp_{p}{c}")
                    nc.vector.memset(t[:], 0.0)
                    nc.sync.dma_start(t[:, :, 1:9, :],
                                      lat_src[p][c * 128:(c + 1) * 128])
                    latp[p, c] = t
                    t = p1.tile([128, 9, COUT], F16, name=f"wts_{p}{c}",
                                tag=f"wts_{p}{c}")
                    nc.sync.dma_start(t[:], wt_src[p][c * 128:(c + 1) * 128])
                    wts[p, c] = t
            # codebook loads queued after the conv inputs: needed ~170us later
            for p in ("h", "l"):
                for d in range(2):
                    nc.sync.dma_start(emb[p, d][:],
                                      aps[f"emb_{p}"][d * 128:(d + 1) * 128, :])
            nc.sync.dma_start(eb2t[:], aps["eb2hl"][:])

            for bc in range(4):          # 8 images -> 512 positions each
                for cc in range(2):      # dout chunk
                    ps = psp.tile([128, 512], F32, name="ps_conv", tag="ps")
                    ps_v = ps[:].rearrange("p (b h w) -> p (b h) w", h=8, w=8)
                    n_mm = len(shifts) * 4 * len(prods)
                    i_mm = 0
                    for ci in range(4):
                        for (dy, dx) in shifts:
                            w0, w1 = max(0, -dx), 8 - max(0, dx)
                            k = (dy + 1) * 3 + (dx + 1)
                            for (pw, pl) in prods:
                                rhs = latp[pl, ci][:, bc * 8:(bc + 1) * 8,
                                                   1 + dy:9 + dy,
                                                   w0 + dx:w1 + dx]
                                nc.tensor.matmul(
                                    ps_v[:, :, w0:w1],
                                    wts[pw, ci][:, k, cc * 128:(cc + 1) * 128],
                                    rhs, start=(i_mm == 0),
                                    stop=(i_mm == n_mm - 1))
                                i_mm += 1
                    sl = slice(bc * 512, (bc + 1) * 512)
                    nc.scalar.copy(flat["h", cc][:, sl], ps[:])
                    nc.vector.tensor_tensor(out=flat["l", cc][:, sl], in0=ps[:],
                                            in1=flat["h", cc][:, sl],
                                            op=mybir.AluOpType.subtract)

        # ---- phase 2: scores + argmax per pos-chunk ----
        scp = ctx.enter_context(tc.tile_pool(name="scp", bufs=2))
        passes = [("h", "h"), ("h", "l"), ("l", "h")]  # (flat, emb)
        for pc in range(NPC):
            psl = slice(pc * 128, (pc + 1) * 128)
            scores = scp.tile([128, K], F32, name="scores", tag="scores")
            for kci in range(NKC):
                ps = psp.tile([128, 512], F32, name="ps_s", tag="ps")
                # -eb2' term first: ones2.T @ eb2hl (K=2), clears the bank
                nc.tensor.matmul(ps[:], ones2[:],
                                 eb2t[:, kci * 512:(kci + 1) * 512],
                                 start=True, stop=False)
                for ip, (pf, pe) in enumerate(passes):
                    for d in range(2):
                        nc.tensor.matmul(
                            ps[:], flat[pf, d][:, psl],
                            emb[pe, d][:, kci * 512:(kci + 1) * 512],
                            start=False,
                            stop=(ip == len(passes) - 1 and d == 1))
                nc.scalar.copy(scores[:, kci * 512:(kci + 1) * 512], ps[:])
            # two-half argmax: each half scans as soon as its 8 chunks are
            # persisted, so the tail after the last matmul is one half-scan.
            m8a = scp.tile([128, 8], F32, name="m8a", tag="m8a")
            nc.vector.max(m8a[:], scores[:, :K // 2])
            mia = scp.tile([128, 8], U32, name="mia", tag="mia")
            nc.vector.max_index(mia[:], m8a[:], scores[:, :K // 2])
            m8b = scp.tile([128, 8], F32, name="m8b", tag="m8b")
            nc.vector.max(m8b[:], scores[:, K // 2:])
            mib = scp.tile([128, 8], U32, name="mib", tag="mib")
            nc.vector.max_index(mib[:], m8b[:], scores[:, K // 2:])
            # merge halves; strict > keeps first-occurrence tie semantics
            gt = scp.tile([128, 1], U32, name="gt", tag="gt")
            nc.vector.tensor_tensor(out=gt[:], in0=m8b[:, 0:1], in1=m8a[:, 0:1],
                                    op=mybir.AluOpType.is_gt)
            mibo = scp.tile([128, 1], U32, name="mibo", tag="mibo")
            nc.vector.tensor_scalar_add(mibo[:], mib[:, 0:1], K // 2)
            nc.vector.select(idx_all[:, pc:pc + 1], gt[:], mibo[:], mia[:, 0:1])

        # ---- output ----
        nc.sync.dma_start(idx_ap.rearrange("(c r) -> r c", r=128),
                          idx_all[:].bitcast(I32))


def _get_nc(n_iters=1):
    key = n_iters
    if key not in _CACHE:
        _CACHE[key] = build_kernel(n_iters)
    return _CACHE[key]


def _split16(a):
    h = a.astype(np.float16)
    l = (a.astype(np.float32) - h.astype(np.float32)).astype(np.float16)
    return h, l


def prepare_inputs(latent, conv_w, conv_b, lin_w, lin_b, emb):
    """Host-side prep: fuse linear into conv weights, fold biases into eb2,
    fp16 hi/lo splits, per-core batch shards."""
    # fused weights: W'[ky,kx,ci,do] = sum_co lin_w[do,co] * conv_w[co,ci,ky,kx]
    wf = np.einsum("dc,cxyz->yzxd", lin_w.astype(np.float64),
                   conv_w.astype(np.float64))
    wt_h, wt_l = _split16(np.ascontiguousarray(wf))
    embT = np.ascontiguousarray(emb.T)                            # [d, K]
    emb_h, emb_l = _split16(embT)
    # biases: x = y + c with c = lin_w @ conv_b + lin_b
    c = lin_w.astype(np.float64) @ conv_b.astype(np.float64) + lin_b.astype(np.float64)
    eb2 = (0.5 * np.sum(emb.astype(np.float64) ** 2, axis=1)
           - emb.astype(np.float64) @ c).astype(np.float32)
    eb2_h, eb2_l = _split16(-eb2)
    eb2hl = np.stack([eb2_h, eb2_l])
    in_maps = []
    for ci in range(NCORES):
        lat_h, lat_l = _split16(latent[ci * BL:(ci + 1) * BL])
        in_maps.append({
            "lat_h": lat_h, "lat_l": lat_l,
            "wt_h": wt_h, "wt_l": wt_l,
            "emb_h": emb_h, "emb_l": emb_l, "eb2hl": eb2hl,
        })
    return in_maps


def kernel(latent, conv_w, conv_b, lin_w, lin_b, emb):
    latent = np.asarray(latent, dtype=np.float32)
    conv_w = np.asarray(conv_w, dtype=np.float32)
    conv_b = np.asarray(conv_b, dtype=np.float32)
    lin_w = np.asarray(lin_w, dtype=np.float32)
    lin_b = np.asarray(lin_b, dtype=np.float32)
    emb = np.asarray(emb, dtype=np.float32)

    nc = _get_nc(1)
    in_maps = prepare_inputs(latent, conv_w, conv_b, lin_w, lin_b, emb)
    res = run_bass_kernel_spmd(nc, in_maps, core_ids=list(range(NCORES)))
    out = np.concatenate([res.results[c]["idx"] for c in range(NCORES)])
    return out.reshape(-1, 64).astype(np.int32)


# revision 17
# speedup vs baseline: 1.0199x; 1.0199x over previous
"""VQ codebook nearest-neighbor encode kernel for Trainium2 (8 NeuronCores).

Pipeline (matches the reference nn.Module):
  conv3x3(SAME) -> positionwise linear -> argmin_k |x - e_k|^2.

Algebraic folds (host-side, fp64):
  * The positionwise linear commutes with the conv:
      L @ conv(W, x) == conv(L @ W, x)
    so the linear layer is fused into the conv weights and disappears.
  * argmin_k |x-e_k|^2 = argmax_k (x.e_k - 0.5|e_k|^2); the conv/linear
    biases shift every x by one constant vector c, so they fold into the
    per-codeword table eb2'_k = 0.5|e_k|^2 - c.e_k. No on-chip bias adds.

On-chip compute:
  * Conv as 9 shifted matmuls accumulating in PSUM (H padded to 10 in SBUF,
    4D strided moving APs, per-element has_written semantics with the full
    (0,0) shift first). cin-chunk-outer loop over all 8 PSUM banks so
    matmuls start as soon as the first latent chunk lands.
  * Scores: per 512-codeword chunk, first a K=2 matmul injects -eb2'
    (ones x [eb2_hi; eb2_lo]), then 6 product matmuls accumulate.
  * All matmuls are fp16 hi/lo 3-pass products (a.b ~= ah.bh + ah.bl + al.bh,
    fp32 PSUM accumulation): ~2^-24 relative error at 1 cycle/row (vs 4 for
    native fp32), so the argmin matches the fp32 reference exactly.
  * ScalarE persists score chunks PSUM->SBUF; VectorE finds the argmax with
    max8 + max_index (first-occurrence ties, like jnp.argmin).

Sharding: data-parallel over batch. Each core processes 32 images (2048
positions) end-to-end with a full copy of the weights and codebook. No
collectives; the host concatenates the 8 index shards.
"""

import sys

sys.path.insert(0, "/opt/trn_rl_repo")

import numpy as np

import concourse.bass as bass  # noqa: F401  (registers engines)
import concourse.tile as tile
from concourse import bacc, mybir
from concourse.bass_utils import run_bass_kernel_spmd

F16 = mybir.dt.float16
F32 = mybir.dt.float32
U32 = mybir.dt.uint32
I32 = mybir.dt.int32

B, CIN, COUT, H, W, K = 256, 512, 256, 8, 8, 8192
NCORES = 8
BL = B // NCORES          # images per core (32)
NPOS = BL * H * W         # positions per core (2048)
NPC = NPOS // 128         # pos-chunks per core (16)
NKC = K // 512            # codebook chunks (16)

_CACHE = {}


def build_kernel(n_iters: int = 1):
    nc = bacc.Bacc("TRN2", target_bir_lowering=False, debug=False,
                   num_devices=NCORES)
    aps = {}
    for nm, shape, dt in [
        ("lat_h", [BL, CIN, H, W], F16), ("lat_l", [BL, CIN, H, W], F16),
        ("wt_h", [3, 3, CIN, COUT], F16), ("wt_l", [3, 3, CIN, COUT], F16),
        ("emb_h", [COUT, K], F16), ("emb_l", [COUT, K], F16),
        ("eb2hl", [2, K], F16),
    ]:
        aps[nm] = nc.dram_tensor(nm, shape, dt, kind="ExternalInput").ap()
    idx_ap = nc.dram_tensor("idx", [NPOS], I32, kind="ExternalOutput").ap()

    with tile.TileContext(nc) as tc:
        for _ in range(n_iters):
            _emit_body(nc, tc, aps, idx_ap)
    nc.compile()
    return nc


def _emit_body(nc, tc, aps, idx_ap):
    from contextlib import ExitStack
    ctx = ExitStack()
    with ctx:
        glob = ctx.enter_context(tc.tile_pool(name="glob", bufs=1))
        psp = ctx.enter_context(tc.tile_pool(name="ps", bufs=8, space="PSUM"))

        # ---- whole-kernel-lifetime tiles ----
        emb = {}
        for p in ("h", "l"):
            for d in range(2):
                emb[p, d] = glob.tile([128, K], F16, name=f"emb_{p}{d}",
                                      tag=f"emb_{p}{d}")
        eb2t = glob.tile([2, K], F16, name="eb2t")
        ones2 = glob.tile([2, 128], F16, name="ones2")
        nc.vector.memset(ones2[:], 1.0)

        flat = {}   # (p, dout_chunk) -> [128, NPOS] fp16 fused conv+linear out
        for p in ("h", "l"):
            for d in range(2):
                flat[p, d] = glob.tile([128, NPOS], F16, name=f"flat_{p}{d}",
                                       tag=f"flat_{p}{d}")
        idx_all = glob.tile([128, NPC], U32, name="idx_all")

        # ---- phase 1: fused conv (9 shifted matmuls, H padded to 10) ----
        shifts = [(0, 0)] + [(dy, dx) for dy in (-1, 0, 1) for dx in (-1, 0, 1)
                             if (dy, dx) != (0, 0)]
        prods = [("h", "h"), ("h", "l"), ("l", "h")]  # (weight, latent)
        with tc.tile_pool(name="p1", bufs=1) as p1:
            latp, wts = {}, {}
            lat_src = {p: aps[f"lat_{p}"].rearrange("b c h w -> c b h w")
                       for p in ("h", "l")}
            wt_src = {p: aps[f"wt_{p}"].rearrange("ky kx ci co -> ci (ky kx) co")
                      for p in ("h", "l")}
            for c in range(4):          # per-cin-chunk loads, in loop order
                for p in ("h", "l"):
                    t = p1.tile([128, BL, 10, 8], F16, name=f"latp_{p}{c}",
                                tag=f"latp_{p}{c}")
                    nc.vector.memset(t[:], 0.0)
                    nc.sync.dma_start(t[:, :, 1:9, :],
                                      lat_src[p][c * 128:(c + 1) * 128])
                    latp[p, c] = t
                    t = p1.tile([128, 9, COUT], F16, name=f"wts_{p}{c}",
                                tag=f"wts_{p}{c}")
                    nc.sync.dma_start(t[:], wt_src[p][c * 128:(c + 1) * 128])
                    wts[p, c] = t
            # codebook loads queued after the conv inputs: needed ~170us later
            for p in ("h", "l"):
                for d in range(2):
                    nc.sync.dma_start(emb[p, d][:],
                                      aps[f"emb_{p}"][d * 128:(d + 1) * 128, :])
            nc.sync.dma_start(eb2t[:], aps["eb2hl"][:])

            for bc in range(4):          # 8 images -> 512 positions each
                for cc in range(2):      # dout chunk
                    ps = psp.tile([128, 512], F32, name="ps_conv", tag="ps")
                    ps_v = ps[:].rearrange("p (b h w) -> p (b h) w", h=8, w=8)
                    n_mm = len(shifts) * 4 * len(prods)
                    i_mm = 0
                    for ci in range(4):
                        for (dy, dx) in shifts:
                            w0, w1 = max(0, -dx), 8 - max(0, dx)
                            k = (dy + 1) * 3 + (dx + 1)
                            for (pw, pl) in prods:
                                rhs = latp[pl, ci][:, bc * 8:(bc + 1) * 8,
                                                   1 + dy:9 + dy,
                                                   w0 + dx:w1 + dx]
                                nc.tensor.matmul(
                                    ps_v[:, :, w0:w1],
                                    wts[pw, ci][:, k, cc * 128:(cc + 1) * 128],
                                    rhs, start=(i_mm == 0),
                                    stop=(i_mm == n_mm - 1))
                                i_mm += 1
                    sl = slice(bc * 512, (bc + 1) * 512)
                    nc.scalar.copy(flat["h", cc][:, sl], ps[:])
                    nc.vector.tensor_tensor(out=flat["l", cc][:, sl], in0=ps[:],
                                            in1=flat["h", cc][:, sl],
                                            op=mybir.AluOpType.subtract)

        # ---- phase 2: scores + argmax per pos-chunk ----
        scp = ctx.enter_context(tc.tile_pool(name="scp", bufs=2))
        passes = [("h", "h"), ("h", "l"), ("l", "h")]  # (flat, emb)
        for pc in range(NPC):
            psl = slice(pc * 128, (pc + 1) * 128)
            scores = scp.tile([128, K], F32, name="scores", tag="scores")
            for kci in range(NKC):
                ps = psp.tile([128, 512], F32, name="ps_s", tag="ps")
                # -eb2' term first: ones2.T @ eb2hl (K=2), clears the bank
                nc.tensor.matmul(ps[:], ones2[:],
                                 eb2t[:, kci * 512:(kci + 1) * 512],
                                 start=True, stop=False)
                for ip, (pf, pe) in enumerate(passes):
                    for d in range(2):
                        nc.tensor.matmul(
                            ps[:], flat[pf, d][:, psl],
                            emb[pe, d][:, kci * 512:(kci + 1) * 512],
                            start=False,
                            stop=(ip == len(passes) - 1 and d == 1))
                nc.scalar.copy(scores[:, kci * 512:(kci + 1) * 512], ps[:])
            # two-half argmax: each half scans as soon as its 8 chunks are
            # persisted, so the tail after the last matmul is one half-scan.
            m8a = scp.tile([128, 8], F32, name="m8a", tag="m8a")
            nc.vector.max(m8a[:], scores[:, :K // 2])
            mia = scp.tile([128, 8], U32, name="mia", tag="mia")
            nc.vector.max_index(mia[:], m8a[:], scores[:, :K // 2])
            m8b = scp.tile([128, 8], F32, name="m8b", tag="m8b")
            nc.vector.max(m8b[:], scores[:, K // 2:])
            mib = scp.tile([128, 8], U32, name="mib", tag="mib")
            nc.vector.max_index(mib[:], m8b[:], scores[:, K // 2:])
            # merge halves; strict > keeps first-occurrence tie semantics
            gt = scp.tile([128, 1], U32, name="gt", tag="gt")
            nc.vector.tensor_tensor(out=gt[:], in0=m8b[:, 0:1], in1=m8a[:, 0:1],
                                    op=mybir.AluOpType.is_gt)
            mibo = scp.tile([128, 1], U32, name="mibo", tag="mibo")
            nc.vector.tensor_scalar_add(mibo[:], mib[:, 0:1], K // 2)
            nc.vector.select(idx_all[:, pc:pc + 1], gt[:], mibo[:], mia[:, 0:1])

        # ---- output ----
        nc.sync.dma_start(idx_ap.rearrange("(c r) -> r c", r=128),
                          idx_all[:].bitcast(I32))


def _get_nc(n_iters=1):
    key = n_iters
    if key not in _CACHE:
        _CACHE[key] = build_kernel(n_iters)
    return _CACHE[key]


def _split16(a):
    h = a.astype(np.float16)
    l = (a.astype(np.float32) - h.astype(np.float32)).astype(np.float16)
    return h, l


def prepare_inputs(latent, conv_w, conv_b, lin_w, lin_b, emb):
    """Host-side prep: fuse linear into conv weights, fold biases into eb2,
    fp16 hi/lo splits, per-core batch shards."""
    # fused weights: W'[ky,kx,ci,do] = sum_co lin_w[do,co] * conv_w[co,ci,ky,kx]
    wf = np.einsum("dc,cxyz->yzxd", lin_w.astype(np.float64),
                   conv_w.astype(np.float64))
    wt_h, wt_l = _split16(np.ascontiguousarray(wf))
    embT = np.ascontiguousarray(emb.T)                            # [d, K]
    emb_h, emb_l = _split16(embT)
    # biases: x = y + c with c = lin_w @ conv_b + lin_b
    c = lin_w.astype(np.float64) @ conv_b.astype(np.float64) + lin_b.astype(np.float64)
    eb2 = (0.5 * np.sum(emb.astype(np.float64) ** 2, axis=1)
           - emb.astype(np.float64) @ c).astype(np.float32)
    eb2_h, eb2_l = _split16(-eb2)
    eb2hl = np.stack([eb2_h, eb2_l])
    in_maps = []
    for ci in range(NCORES):
        lat_h, lat_l = _split16(latent[ci * BL:(ci + 1) * BL])
        in_maps.append({
            "lat_h": lat_h, "lat_l": lat_l,
            "wt_h": wt_h, "wt_l": wt_l,
            "emb_h": emb_h, "emb_l": emb_l, "eb2hl": eb2hl,
        })
    return in_maps


def kernel(latent, conv_w, conv_b, lin_w, lin_b, emb):
    latent = np.asarray(latent, dtype=np.float32)
    conv_w = np.asarray(conv_w, dtype=np.float32)
    conv_b = np.asarray(conv_b, dtype=np.float32)
    lin_w = np.asarray(lin_w, dtype=np.float32)
    lin_b = np.asarray(lin_b, dtype=np.float32)
    emb = np.asarray(emb, dtype=np.float32)

    nc = _get_nc(1)
    in_maps = prepare_inputs(latent, conv_w, conv_b, lin_w, lin_b, emb)
    res = run_bass_kernel_spmd(nc, in_maps, core_ids=list(range(NCORES)))
    out = np.concatenate([res.results[c]["idx"] for c in range(NCORES)])
    return out.reshape(-1, 64).astype(np.int32)
